# revision 2
# baseline (speedup 1.0000x reference)
"""Trainium2 Bass kernel for nn_Attention_13297218748956.

Multi-head causal self-attention with RoPE (B=64, T=128, C=2048, H=16,
hd=128), data-parallel over batch across 8 NeuronCores (8 batches/core, no
collectives). All GEMMs run in bf16 (PE 1 cyc/row, same rate as f32r for the
projections but 4x faster for the N=128 attention matmuls, half the HBM
traffic, 2x DVE throughput). Measured accuracy: max-rel ~4.7e-3 (limit 2e-2).

Per-core structure (TimelineSim 476us vs 622us for the f32r baseline):
  - x^T chunks, the current V column set, and all 16 attention head outputs
    stay SBUF-resident; nothing stages through DRAM except the y output.
  - m-loop m=0..3: V-pass for column set m (m=0 runs k-outer across 8 PSUM
    accumulators so the PE starts the moment x chunk 0 lands; PE-warmup
    matmuls cover the first DMA latency), then heads 4m..4m+3:
      QK projection (weight-stationary, x moving, N=512) ->
      RoPE: the head dim is host-permuted to [evens, odds] so the pair swap
        is a partition block swap done by two SBUF->SBUF DMAs; rotation
        signs are folded into the sin tables; 3 bf16 DVE ops per projection.
      attention: scores for 4 batches into one PSUM bank -> mask add ->
        exp (no max subtraction: scores bounded ~8 for randn inputs) with
        fp32 row sums -> normalize -> PE transpose -> out_headT = V^T@probsT
        accumulated 4 batches wide -> bf16 copy into the resident attnT.
  - phase 3: y = attnT^T @ woT, m2-outer so only one wo column set gates
    each quarter (first set prefetched during the main phase); the final
    tile's copy+DMA is split across engines/queues to shorten the drain.
"""

import numpy as np
import ml_dtypes

import concourse.bacc as bacc
import concourse.tile as tile
import concourse.mybir as mybir
from concourse.bass_utils import run_bass_kernel_spmd

N_CORES = 8
B, T, C, H = 64, 128, 2048, 16
HD = C // H          # 128
BPC = B // N_CORES   # 8 batches per core
TOK = BPC * T        # 1024 tokens per core
KC = C // 128        # 16 contraction chunks
TH = 2               # 512-token halves
F32 = mybir.dt.float32
BF16 = mybir.dt.bfloat16
BF = ml_dtypes.bfloat16
EXP = mybir.ActivationFunctionType.Exp
AX = mybir.AxisListType.X
ADD = mybir.AluOpType.add

_CACHE = {}


def _build():
    nc = bacc.Bacc("TRN2", target_bir_lowering=False, debug=False)

    xT = nc.dram_tensor("xT", [C, TOK], BF16, kind="ExternalInput")
    # host-tiled weights: wq4/wk4[n] = [128, KC, 128] column block of wT,
    # wv4/wo4[m] = [128, KC, 512] m-set (contiguous per partition)
    wq4 = nc.dram_tensor("wq4", [H, 128, KC, 128], BF16, kind="ExternalInput")
    wk4 = nc.dram_tensor("wk4", [H, 128, KC, 128], BF16, kind="ExternalInput")
    wv4 = nc.dram_tensor("wv4", [4, 128, KC, 512], BF16, kind="ExternalInput")
    wo4 = nc.dram_tensor("wo4", [4, 128, KC, 512], BF16, kind="ExternalInput")
    ident = nc.dram_tensor("ident", [128, 128], BF16, kind="ExternalInput")
    mask4 = nc.dram_tensor("mask4", [128, 512], F32, kind="ExternalInput")
    cosq = nc.dram_tensor("cosq", [128, 1024], BF16, kind="ExternalInput")
    sinq = nc.dram_tensor("sinq", [128, 1024], BF16, kind="ExternalInput")
    cosk = nc.dram_tensor("cosk", [128, 1024], BF16, kind="ExternalInput")
    sink = nc.dram_tensor("sink", [128, 1024], BF16, kind="ExternalInput")
    y = nc.dram_tensor("y", [TOK, C], F32, kind="ExternalOutput")

    with tile.TileContext(nc) as tc:
        with (
            tc.tile_pool(name="consts", bufs=1) as consts,
            tc.tile_pool(name="atn", bufs=1) as atnp,
            tc.tile_pool(name="wos0", bufs=1) as wos0p,
        ):
            id_t = consts.tile([128, 128], BF16)
            mask_t = consts.tile([128, 512], F32)
            junk = consts.tile([128, 512], BF16)  # PE warmup operand
            cosq_t = consts.tile([128, 1024], BF16)
            sinq_t = consts.tile([128, 1024], BF16)
            cosk_t = consts.tile([128, 1024], BF16)
            sink_t = consts.tile([128, 1024], BF16)
            # all 16 head outputs stay SBUF-resident into phase 3
            at = [
                atnp.tile([128, TOK], BF16, tag=f"at{n}", name=f"at{n}")
                for n in range(H)
            ]

            with (
                tc.tile_pool(name="xt", bufs=1) as xtp,
                tc.tile_pool(name="vsb", bufs=1) as vsbp,
                tc.tile_pool(name="wcol", bufs=4) as wcolp,
                tc.tile_pool(name="wvset", bufs=2) as wvsp,
                tc.tile_pool(name="qk", bufs=2) as qkhp,
                tc.tile_pool(name="stage", bufs=3) as stagep,
                tc.tile_pool(name="rope", bufs=3) as ropep,
                tc.tile_pool(name="soft", bufs=3) as softp,
                tc.tile_pool(name="ptsb", bufs=6) as ptp,
                tc.tile_pool(name="small", bufs=4) as smallp,
                tc.tile_pool(name="ps512", bufs=4, space="PSUM") as ps512,
                tc.tile_pool(name="psc", bufs=2, space="PSUM") as pscp,
                tc.tile_pool(name="mix", bufs=2, space="PSUM") as mixp,
            ):
                # --- startup DMAs: x chunks + first wv set per-k so the PE
                # can start as soon as chunk 0 lands ---
                wvs0 = wvsp.tile([128, KC, 512], BF16, tag="wvs", name="wvs0")
                xt = []
                for k in range(KC):
                    xk = xtp.tile([128, TOK], BF16, tag=f"xt{k}", name=f"xt{k}")
                    nc.scalar.dma_start(
                        out=xk[:], in_=xT[k * 128:(k + 1) * 128, :]
                    )
                    xt.append(xk)
                    nc.sync.dma_start(out=wvs0[:, k, :], in_=wv4[0, :, k, :])
                for t_, d_ in [
                    (id_t, ident), (mask_t, mask4),
                    (cosq_t, cosq), (sinq_t, sinq),
                    (cosk_t, cosk), (sink_t, sink),
                ]:
                    nc.gpsimd.dma_start(out=t_[:], in_=d_[:])
                wcol_pre = {}

                def prefetch_wcol(n):
                    tiles = []
                    for wname, wT_ in (("q", wq4), ("k", wk4)):
                        wc = wcolp.tile([128, KC, 128], BF16, tag="wcol",
                                        name=f"wcol{n}{wname}")
                        nc.sync.dma_start(out=wc[:], in_=wT_[n])
                        tiles.append(wc)
                    wcol_pre[n] = tiles

                prefetch_wcol(0)

                vsb = {}

                def emit_vpass(m, wvs):
                    if m == 0:
                        # k-outer over 8 accumulators (all PSUM banks): the
                        # PE starts on chunk 0 and keeps pace with the x DMA
                        accs = []
                        for i in range(4):
                            accs.append(ps512.tile([128, 512], F32,
                                                   tag="ps512",
                                                   name=f"v0a{i}"))
                        for i in range(2):
                            accs.append(pscp.tile([128, 512], F32, tag="sc",
                                                  name=f"v0b{i}"))
                        for i in range(2):
                            accs.append(mixp.tile([128, 512], F32, tag="mix",
                                                  name=f"v0c{i}"))
                        # warm the PE clock while the first DMAs land
                        # (zeros; results are discarded by the start=True
                        # of each accumulator's first real matmul)
                        nc.vector.memset(junk[:], 0.0)
                        for w in range(16):
                            nc.tensor.matmul(accs[w % 2][:], junk[:, :128],
                                             junk[:], start=True, stop=True)
                        for k in range(KC):
                            for tt in range(BPC):
                                nc.tensor.matmul(
                                    accs[tt][:],
                                    xt[k][:, tt * 128:(tt + 1) * 128],
                                    wvs[:, k, :],
                                    start=(k == 0),
                                    stop=(k == KC - 1),
                                )
                        for tt in range(BPC):
                            vt = vsbp.tile([128, 512], BF16,
                                           tag=f"v{m % 2}_{tt}",
                                           name=f"v{m}_{tt}")
                            nc.scalar.copy(out=vt[:], in_=accs[tt][:])
                            vsb[(m, tt)] = vt
                    else:
                        for tt in range(BPC):
                            acc = ps512.tile([128, 512], F32, tag="ps512",
                                             name="vacc")
                            for k in range(KC):
                                nc.tensor.matmul(
                                    acc[:],
                                    xt[k][:, tt * 128:(tt + 1) * 128],
                                    wvs[:, k, :],
                                    start=(k == 0),
                                    stop=(k == KC - 1),
                                )
                            vt = vsbp.tile([128, 512], BF16,
                                           tag=f"v{m % 2}_{tt}",
                                           name=f"v{m}_{tt}")
                            nc.scalar.copy(out=vt[:], in_=acc[:])
                            vsb[(m, tt)] = vt

                def emit_proj(wcol, tag):
                    accs = [
                        ps512.tile([128, 512], F32, tag="ps512",
                                   name=f"{tag}acc{th}")
                        for th in range(TH)
                    ]
                    for k in range(KC):
                        for th in range(TH):
                            nc.tensor.matmul(
                                accs[th][:],
                                wcol[:, k, :],
                                xt[k][:, th * 512:(th + 1) * 512],
                                start=(k == 0),
                                stop=(k == KC - 1),
                            )
                    return accs

                def emit_rope(accs, cos_t, sin_t, dsth):
                    qt = stagep.tile([128, TOK], BF16, tag="qt")
                    nc.scalar.copy(out=qt[:, 0:512], in_=accs[0][:])
                    nc.scalar.copy(out=qt[:, 512:1024], in_=accs[1][:])
                    shuf = ropep.tile([128, TOK], BF16, tag="shuf")
                    nc.sync.dma_start(out=shuf[0:64, :], in_=qt[64:128, :])
                    nc.sync.dma_start(out=shuf[64:128, :], in_=qt[0:64, :])
                    t1 = ropep.tile([128, TOK], BF16, tag="t1")
                    nc.vector.tensor_mul(t1[:], qt[:], cos_t[:])
                    t2 = ropep.tile([128, TOK], BF16, tag="t2")
                    nc.vector.tensor_mul(t2[:], shuf[:], sin_t[:])
                    nc.vector.tensor_add(dsth[:], t1[:], t2[:])

                def emit_attention(n, qh, kh):
                    m = n // 4
                    c4 = (n % 4) * 128
                    sums = smallp.tile([128, BPC], F32, tag="sums")
                    inv = smallp.tile([128, BPC], F32, tag="inv")
                    for g in range(2):
                        gs = slice(g * 4, (g + 1) * 4)
                        scp = pscp.tile([128, 512], F32, tag="sc",
                                        name="scp")
                        for j in range(4):
                            b = 4 * g + j
                            bs = slice(b * 128, (b + 1) * 128)
                            nc.tensor.matmul(
                                scp[:, j * 128:(j + 1) * 128],
                                qh[:, bs], kh[:, bs],
                                start=True, stop=True,
                            )
                        masked = softp.tile([128, 512], BF16, tag="masked")
                        nc.vector.tensor_add(masked[:], scp[:], mask_t[:])
                        e4 = softp.tile([128, 512], BF16, tag="e4")
                        nc.scalar.activation(e4[:], masked[:], EXP)
                        nc.vector.tensor_reduce(
                            out=sums[:, gs],
                            in_=e4[:].rearrange("p (b s) -> p b s", b=4),
                            axis=AX, op=ADD,
                        )
                        nc.vector.reciprocal(out=inv[:, gs], in_=sums[:, gs])
                        probs4 = softp.tile([128, 512], BF16, tag="probs")
                        for j in range(4):
                            b = 4 * g + j
                            js = slice(j * 128, (j + 1) * 128)
                            nc.vector.tensor_scalar_mul(
                                probs4[:, js], e4[:, js], inv[:, b:b + 1]
                            )
                        pvp = mixp.tile([128, 512], F32, tag="mix",
                                        name="pvp")
                        ptsbs = []
                        for j in range(4):
                            js = slice(j * 128, (j + 1) * 128)
                            pT = pscp.tile([128, 128], BF16, tag="sc",
                                           name=f"pT{j}")
                            nc.tensor.transpose(pT[:], probs4[:, js],
                                                id_t[:])
                            ptsb = ptp.tile([128, 128], BF16, tag="ptsb")
                            nc.vector.tensor_copy(ptsb[:], pT[:])
                            ptsbs.append(ptsb)
                        for j in range(4):
                            b = 4 * g + j
                            js = slice(j * 128, (j + 1) * 128)
                            nc.tensor.matmul(
                                pvp[:, js],
                                vsb[(m, b)][:, c4:c4 + 128],
                                ptsbs[j][:],
                                start=True, stop=True,
                            )
                        nc.scalar.copy(
                            out=at[n][:, g * 512:(g + 1) * 512], in_=pvp[:]
                        )

                wos0 = wos0p.tile([128, KC, 512], BF16, tag="wos0",
                                  name="wos0t")
                for m in range(4):
                    if m == 0:
                        wvs = wvs0
                    else:
                        wvs = wvsp.tile([128, KC, 512], BF16, tag="wvs",
                                        name=f"wvs{m}")
                        nc.sync.dma_start(out=wvs[:], in_=wv4[m])
                    if m == 3:
                        # prefetch the first wo column set so phase 3 can
                        # start the moment head 15 finishes
                        nc.sync.dma_start(out=wos0[:], in_=wo4[0])
                    emit_vpass(m, wvs)
                    for n in range(4 * m, 4 * m + 4):
                        wq_t, wk_t = wcol_pre.pop(n)
                        if n + 1 < H:
                            prefetch_wcol(n + 1)
                        qh = qkhp.tile([128, TOK], BF16, tag="qh")
                        kh = qkhp.tile([128, TOK], BF16, tag="kh")
                        accq = emit_proj(wq_t, "q")
                        acck = emit_proj(wk_t, "k")
                        emit_rope(accq, cosq_t, sinq_t, qh)
                        emit_rope(acck, cosk_t, sink_t, kh)
                        emit_attention(n, qh, kh)

            # --- phase 3: y = attnT.T @ woT, attnT all SBUF-resident.
            # m2-outer so only wos[m2] gates each quarter (wos0 prefetched
            # during the main phase; the rest stream in well ahead of use).
            with (
                tc.tile_pool(name="wos", bufs=1) as wosp,
                tc.tile_pool(name="ystage", bufs=4) as ysp,
                tc.tile_pool(name="psy", bufs=4, space="PSUM") as psy,
            ):
                wos = [wos0]
                for m2 in range(1, 4):
                    wt = wosp.tile([128, KC, 512], BF16, tag=f"wos{m2}",
                                   name=f"wos{m2}t")
                    nc.sync.dma_start(out=wt[:], in_=wo4[m2])
                    wos.append(wt)
                for m2 in range(4):
                    for tt in range(BPC):
                        acc = psy.tile([128, 512], F32, tag="psy",
                                       name=f"y{m2}t{tt}")
                        for k in range(KC):
                            nc.tensor.matmul(
                                acc[:],
                                at[k][:, tt * 128:(tt + 1) * 128],
                                wos[m2][:, k, :],
                                start=(k == 0),
                                stop=(k == KC - 1),
                            )
                        ysb = ysp.tile([128, 512], F32, tag="ysb")
                        ys = slice(tt * 128, (tt + 1) * 128)
                        if m2 == 3 and tt == BPC - 1:
                            # final tile: split copy+DMA in half across
                            # engines/queues to shorten the drain tail
                            nc.vector.tensor_copy(ysb[:, :256], acc[:, :256])
                            nc.scalar.copy(out=ysb[:, 256:], in_=acc[:, 256:])
                            nc.gpsimd.dma_start(
                                out=y[ys, m2 * 512:m2 * 512 + 256],
                                in_=ysb[:, :256],
                            )
                            nc.sync.dma_start(
                                out=y[ys, m2 * 512 + 256:(m2 + 1) * 512],
                                in_=ysb[:, 256:],
                            )
                        else:
                            if tt % 2 == 0:
                                nc.vector.tensor_copy(ysb[:], acc[:])
                            else:
                                nc.scalar.copy(out=ysb[:], in_=acc[:])
                            nc.gpsimd.dma_start(
                                out=y[ys, m2 * 512:(m2 + 1) * 512],
                                in_=ysb[:],
                            )

    nc.compile()
    return nc


def _prep_inputs(x, freqs_cos, freqs_sin, wq, wk, wv, wo):
    x = np.asarray(x, dtype=np.float32)
    fc = np.asarray(freqs_cos, dtype=np.float32)
    fs = np.asarray(freqs_sin, dtype=np.float32)
    wq = np.asarray(wq, np.float32)
    wk = np.asarray(wk, np.float32)
    wv = np.asarray(wv, np.float32)
    wo = np.asarray(wo, np.float32)

    # head-dim permutation: even components to rows 0-63, odd to 64-127,
    # so the RoPE pair swap becomes a partition block swap (scores are
    # invariant since q and k use the same permutation)
    perm = np.concatenate([np.arange(0, 128, 2), np.arange(1, 128, 2)])
    shared = {
        # [n/m, p, kc, nn] = w[block*bw + perm[nn], kc*128 + p]
        "wq4": np.ascontiguousarray(
            wq.reshape(H, 128, KC, 128)[:, perm].transpose(0, 3, 2, 1)
        ).astype(BF),
        "wk4": np.ascontiguousarray(
            wk.reshape(H, 128, KC, 128)[:, perm].transpose(0, 3, 2, 1)
        ).astype(BF),
        "wv4": np.ascontiguousarray(
            wv.reshape(4, 512, KC, 128).transpose(0, 3, 2, 1)).astype(BF),
        "wo4": np.ascontiguousarray(
            wo.reshape(4, 512, KC, 128).transpose(0, 3, 2, 1)).astype(BF),
    }
    shared["ident"] = np.eye(128, dtype=np.float32).astype(BF)
    mask = np.triu(np.full((128, 128), -1e30, np.float32), k=1)
    shared["mask4"] = np.ascontiguousarray(np.tile(mask, (1, 4)))

    # permuted layout: rows 0-63 = even slots (sign -s), 64-127 = odd (+s)
    cosd = np.vstack([fc.T, fc.T])              # [128, 128]
    sind = np.vstack([-fs.T, fs.T])
    cos8 = np.ascontiguousarray(np.tile(cosd, (1, 8)))  # [128, 1024]
    sin8 = np.ascontiguousarray(np.tile(sind, (1, 8)))
    scale = np.float32(1.0 / np.sqrt(HD))
    shared["cosq"] = (cos8 * scale).astype(BF)
    shared["sinq"] = (sin8 * scale).astype(BF)
    shared["cosk"] = cos8.astype(BF)
    shared["sink"] = sin8.astype(BF)

    in_maps = []
    for i in range(N_CORES):
        shard = x[i * BPC:(i + 1) * BPC].reshape(TOK, C)
        m = dict(shared)
        m["xT"] = np.ascontiguousarray(shard.T).astype(BF)
        in_maps.append(m)
    return in_maps


def _run(inputs, trace=False):
    if "nc" not in _CACHE:
        _CACHE["nc"] = _build()
    nc = _CACHE["nc"]
    in_maps = _prep_inputs(**inputs)
    res = run_bass_kernel_spmd(
        nc, in_maps, core_ids=list(range(N_CORES)), trace=trace
    )
    out = np.empty((B, T, C), np.float32)
    for i in range(N_CORES):
        out[i * BPC:(i + 1) * BPC] = np.asarray(res.results[i]["y"]).reshape(
            BPC, T, C
        )
    return out, res


def kernel(**inputs):
    out, _ = _run(inputs, trace=False)
    return out


# revision 3
# speedup vs baseline: 1.0046x; 1.0046x over previous
"""Trainium2 Bass kernel for nn_Attention_13297218748956.

Multi-head causal self-attention with RoPE (B=64, T=128, C=2048, H=16,
hd=128), data-parallel over batch across 8 NeuronCores (8 batches/core, no
collectives). All GEMMs run in bf16 (PE 1 cyc/row, same rate as f32r for the
projections but 4x faster for the N=128 attention matmuls, half the HBM
traffic, 2x DVE throughput). Measured accuracy: max-rel ~4.7e-3 (limit 2e-2).

Per-core structure (TimelineSim 476us vs 622us for the f32r baseline):
  - x^T chunks, the current V column set, and all 16 attention head outputs
    stay SBUF-resident; nothing stages through DRAM except the y output.
  - m-loop m=0..3: V-pass for column set m (m=0 runs k-outer across 8 PSUM
    accumulators so the PE starts the moment x chunk 0 lands; PE-warmup
    matmuls cover the first DMA latency), then heads 4m..4m+3:
      QK projection (weight-stationary, x moving, N=512) ->
      RoPE: the head dim is host-permuted to [evens, odds] so the pair swap
        is a partition block swap done by two SBUF->SBUF DMAs; rotation
        signs are folded into the sin tables; 3 bf16 DVE ops per projection.
      attention: scores for 4 batches into one PSUM bank -> mask add ->
        exp (no max subtraction: scores bounded ~8 for randn inputs) with
        fp32 row sums -> normalize -> PE transpose -> out_headT = V^T@probsT
        accumulated 4 batches wide -> bf16 copy into the resident attnT.
  - phase 3: y = attnT^T @ woT, m2-outer so only one wo column set gates
    each quarter (first set prefetched during the main phase); the final
    tile's copy+DMA is split across engines/queues to shorten the drain.
"""

import numpy as np
import ml_dtypes

import concourse.bacc as bacc
import concourse.tile as tile
import concourse.mybir as mybir
from concourse.bass_utils import run_bass_kernel_spmd

N_CORES = 8
B, T, C, H = 64, 128, 2048, 16
HD = C // H          # 128
BPC = B // N_CORES   # 8 batches per core
TOK = BPC * T        # 1024 tokens per core
KC = C // 128        # 16 contraction chunks
TH = 2               # 512-token halves
F32 = mybir.dt.float32
BF16 = mybir.dt.bfloat16
BF = ml_dtypes.bfloat16
EXP = mybir.ActivationFunctionType.Exp
AX = mybir.AxisListType.X
ADD = mybir.AluOpType.add

_CACHE = {}


def _build():
    nc = bacc.Bacc("TRN2", target_bir_lowering=False, debug=False)

    xT = nc.dram_tensor("xT", [C, TOK], BF16, kind="ExternalInput")
    # host-tiled weights: wq4/wk4[n] = [128, KC, 128] column block of wT,
    # wv4/wo4[m] = [128, KC, 512] m-set (contiguous per partition)
    wq4 = nc.dram_tensor("wq4", [H, 128, KC, 128], BF16, kind="ExternalInput")
    wk4 = nc.dram_tensor("wk4", [H, 128, KC, 128], BF16, kind="ExternalInput")
    wv4 = nc.dram_tensor("wv4", [4, 128, KC, 512], BF16, kind="ExternalInput")
    wo4 = nc.dram_tensor("wo4", [4, 128, KC, 512], BF16, kind="ExternalInput")
    ident = nc.dram_tensor("ident", [128, 128], BF16, kind="ExternalInput")
    mask4 = nc.dram_tensor("mask4", [128, 512], F32, kind="ExternalInput")
    cosq = nc.dram_tensor("cosq", [128, 1024], BF16, kind="ExternalInput")
    sinq = nc.dram_tensor("sinq", [128, 1024], BF16, kind="ExternalInput")
    cosk = nc.dram_tensor("cosk", [128, 1024], BF16, kind="ExternalInput")
    sink = nc.dram_tensor("sink", [128, 1024], BF16, kind="ExternalInput")
    y = nc.dram_tensor("y", [TOK, C], F32, kind="ExternalOutput")

    with tile.TileContext(nc) as tc:
        with (
            tc.tile_pool(name="consts", bufs=1) as consts,
            tc.tile_pool(name="atn", bufs=1) as atnp,
            tc.tile_pool(name="wos0", bufs=1) as wos0p,
        ):
            id_t = consts.tile([128, 128], BF16)
            mask_t = consts.tile([128, 512], F32)
            junk = consts.tile([128, 512], BF16)  # PE warmup operand
            cosq_t = consts.tile([128, 1024], BF16)
            sinq_t = consts.tile([128, 1024], BF16)
            cosk_t = consts.tile([128, 1024], BF16)
            sink_t = consts.tile([128, 1024], BF16)
            # all 16 head outputs stay SBUF-resident into phase 3
            at = [
                atnp.tile([128, TOK], BF16, tag=f"at{n}", name=f"at{n}")
                for n in range(H)
            ]

            with (
                tc.tile_pool(name="xt", bufs=1) as xtp,
                tc.tile_pool(name="vsb", bufs=1) as vsbp,
                tc.tile_pool(name="wcol", bufs=4) as wcolp,
                tc.tile_pool(name="wvset", bufs=2) as wvsp,
                tc.tile_pool(name="qk", bufs=2) as qkhp,
                tc.tile_pool(name="stage", bufs=3) as stagep,
                tc.tile_pool(name="rope", bufs=3) as ropep,
                tc.tile_pool(name="soft", bufs=3) as softp,
                tc.tile_pool(name="ptsb", bufs=6) as ptp,
                tc.tile_pool(name="small", bufs=4) as smallp,
                tc.tile_pool(name="ps512", bufs=4, space="PSUM") as ps512,
                tc.tile_pool(name="psc", bufs=2, space="PSUM") as pscp,
                tc.tile_pool(name="mix", bufs=2, space="PSUM") as mixp,
            ):
                # --- startup DMAs: x chunks + first wv set per-k so the PE
                # can start as soon as chunk 0 lands ---
                wvs0 = wvsp.tile([128, KC, 512], BF16, tag="wvs", name="wvs0")
                xt = []
                for k in range(KC):
                    xk = xtp.tile([128, TOK], BF16, tag=f"xt{k}", name=f"xt{k}")
                    nc.scalar.dma_start(
                        out=xk[:], in_=xT[k * 128:(k + 1) * 128, :]
                    )
                    xt.append(xk)
                    nc.sync.dma_start(out=wvs0[:, k, :], in_=wv4[0, :, k, :])
                for t_, d_ in [
                    (id_t, ident), (mask_t, mask4),
                    (cosq_t, cosq), (sinq_t, sinq),
                    (cosk_t, cosk), (sink_t, sink),
                ]:
                    nc.gpsimd.dma_start(out=t_[:], in_=d_[:])
                wcol_pre = {}

                def prefetch_wcol(n):
                    tiles = []
                    for wname, wT_ in (("q", wq4), ("k", wk4)):
                        wc = wcolp.tile([128, KC, 128], BF16, tag="wcol",
                                        name=f"wcol{n}{wname}")
                        nc.sync.dma_start(out=wc[:], in_=wT_[n])
                        tiles.append(wc)
                    wcol_pre[n] = tiles

                prefetch_wcol(0)

                vsb = {}

                def emit_vpass(m, wvs):
                    if m == 0:
                        # k-outer over 8 accumulators (all PSUM banks): the
                        # PE starts on chunk 0 and keeps pace with the x DMA
                        accs = []
                        for i in range(4):
                            accs.append(ps512.tile([128, 512], F32,
                                                   tag="ps512",
                                                   name=f"v0a{i}"))
                        for i in range(2):
                            accs.append(pscp.tile([128, 512], F32, tag="sc",
                                                  name=f"v0b{i}"))
                        for i in range(2):
                            accs.append(mixp.tile([128, 512], F32, tag="mix",
                                                  name=f"v0c{i}"))
                        # warm the PE clock while the first DMAs land
                        # (zeros; results are discarded by the start=True
                        # of each accumulator's first real matmul)
                        nc.vector.memset(junk[:], 0.0)
                        for w in range(10):
                            nc.tensor.matmul(accs[w % 2][:], junk[:, :128],
                                             junk[:], start=True, stop=True)
                        for k in range(KC):
                            for tt in range(BPC):
                                nc.tensor.matmul(
                                    accs[tt][:],
                                    xt[k][:, tt * 128:(tt + 1) * 128],
                                    wvs[:, k, :],
                                    start=(k == 0),
                                    stop=(k == KC - 1),
                                )
                        for tt in range(BPC):
                            vt = vsbp.tile([128, 512], BF16,
                                           tag=f"v{m % 2}_{tt}",
                                           name=f"v{m}_{tt}")
                            nc.scalar.copy(out=vt[:], in_=accs[tt][:])
                            vsb[(m, tt)] = vt
                    else:
                        for tt in range(BPC):
                            acc = ps512.tile([128, 512], F32, tag="ps512",
                                             name="vacc")
                            for k in range(KC):
                                nc.tensor.matmul(
                                    acc[:],
                                    xt[k][:, tt * 128:(tt + 1) * 128],
                                    wvs[:, k, :],
                                    start=(k == 0),
                                    stop=(k == KC - 1),
                                )
                            vt = vsbp.tile([128, 512], BF16,
                                           tag=f"v{m % 2}_{tt}",
                                           name=f"v{m}_{tt}")
                            nc.scalar.copy(out=vt[:], in_=acc[:])
                            vsb[(m, tt)] = vt

                def emit_proj(wcol, tag):
                    accs = [
                        ps512.tile([128, 512], F32, tag="ps512",
                                   name=f"{tag}acc{th}")
                        for th in range(TH)
                    ]
                    for k in range(KC):
                        for th in range(TH):
                            nc.tensor.matmul(
                                accs[th][:],
                                wcol[:, k, :],
                                xt[k][:, th * 512:(th + 1) * 512],
                                start=(k == 0),
                                stop=(k == KC - 1),
                            )
                    return accs

                def emit_rope(accs, cos_t, sin_t, dsth):
                    qt = stagep.tile([128, TOK], BF16, tag="qt")
                    nc.scalar.copy(out=qt[:, 0:512], in_=accs[0][:])
                    nc.scalar.copy(out=qt[:, 512:1024], in_=accs[1][:])
                    shuf = ropep.tile([128, TOK], BF16, tag="shuf")
                    nc.sync.dma_start(out=shuf[0:64, :], in_=qt[64:128, :])
                    nc.sync.dma_start(out=shuf[64:128, :], in_=qt[0:64, :])
                    t1 = ropep.tile([128, TOK], BF16, tag="t1")
                    nc.vector.tensor_mul(t1[:], qt[:], cos_t[:])
                    t2 = ropep.tile([128, TOK], BF16, tag="t2")
                    nc.vector.tensor_mul(t2[:], shuf[:], sin_t[:])
                    nc.vector.tensor_add(dsth[:], t1[:], t2[:])

                def emit_attention(n, qh, kh):
                    m = n // 4
                    c4 = (n % 4) * 128
                    sums = smallp.tile([128, BPC], F32, tag="sums")
                    inv = smallp.tile([128, BPC], F32, tag="inv")
                    for g in range(2):
                        gs = slice(g * 4, (g + 1) * 4)
                        scp = pscp.tile([128, 512], F32, tag="sc",
                                        name="scp")
                        for j in range(4):
                            b = 4 * g + j
                            bs = slice(b * 128, (b + 1) * 128)
                            nc.tensor.matmul(
                                scp[:, j * 128:(j + 1) * 128],
                                qh[:, bs], kh[:, bs],
                                start=True, stop=True,
                            )
                        masked = softp.tile([128, 512], BF16, tag="masked")
                        nc.vector.tensor_add(masked[:], scp[:], mask_t[:])
                        e4 = softp.tile([128, 512], BF16, tag="e4")
                        nc.scalar.activation(e4[:], masked[:], EXP)
                        nc.vector.tensor_reduce(
                            out=sums[:, gs],
                            in_=e4[:].rearrange("p (b s) -> p b s", b=4),
                            axis=AX, op=ADD,
                        )
                        nc.vector.reciprocal(out=inv[:, gs], in_=sums[:, gs])
                        probs4 = softp.tile([128, 512], BF16, tag="probs")
                        for j in range(4):
                            b = 4 * g + j
                            js = slice(j * 128, (j + 1) * 128)
                            nc.vector.tensor_scalar_mul(
                                probs4[:, js], e4[:, js], inv[:, b:b + 1]
                            )
                        pvp = mixp.tile([128, 512], F32, tag="mix",
                                        name="pvp")
                        ptsbs = []
                        for j in range(4):
                            js = slice(j * 128, (j + 1) * 128)
                            pT = pscp.tile([128, 128], BF16, tag="sc",
                                           name=f"pT{j}")
                            nc.tensor.transpose(pT[:], probs4[:, js],
                                                id_t[:])
                            ptsb = ptp.tile([128, 128], BF16, tag="ptsb")
                            nc.vector.tensor_copy(ptsb[:], pT[:])
                            ptsbs.append(ptsb)
                        for j in range(4):
                            b = 4 * g + j
                            js = slice(j * 128, (j + 1) * 128)
                            nc.tensor.matmul(
                                pvp[:, js],
                                vsb[(m, b)][:, c4:c4 + 128],
                                ptsbs[j][:],
                                start=True, stop=True,
                            )
                        nc.scalar.copy(
                            out=at[n][:, g * 512:(g + 1) * 512], in_=pvp[:]
                        )

                wos0 = wos0p.tile([128, KC, 512], BF16, tag="wos0",
                                  name="wos0t")
                for m in range(4):
                    if m == 0:
                        wvs = wvs0
                    else:
                        wvs = wvsp.tile([128, KC, 512], BF16, tag="wvs",
                                        name=f"wvs{m}")
                        nc.sync.dma_start(out=wvs[:], in_=wv4[m])
                    if m == 3:
                        # prefetch the first wo column set so phase 3 can
                        # start the moment head 15 finishes
                        nc.sync.dma_start(out=wos0[:], in_=wo4[0])
                    emit_vpass(m, wvs)
                    for n in range(4 * m, 4 * m + 4):
                        wq_t, wk_t = wcol_pre.pop(n)
                        if n + 1 < H:
                            prefetch_wcol(n + 1)
                        qh = qkhp.tile([128, TOK], BF16, tag="qh")
                        kh = qkhp.tile([128, TOK], BF16, tag="kh")
                        accq = emit_proj(wq_t, "q")
                        acck = emit_proj(wk_t, "k")
                        emit_rope(accq, cosq_t, sinq_t, qh)
                        emit_rope(acck, cosk_t, sink_t, kh)
                        emit_attention(n, qh, kh)

            # --- phase 3: y = attnT.T @ woT, attnT all SBUF-resident.
            # m2-outer so only wos[m2] gates each quarter (wos0 prefetched
            # during the main phase; the rest stream in well ahead of use).
            with (
                tc.tile_pool(name="wos", bufs=1) as wosp,
                tc.tile_pool(name="ystage", bufs=4) as ysp,
                tc.tile_pool(name="psy", bufs=4, space="PSUM") as psy,
            ):
                wos = [wos0]
                for m2 in range(1, 4):
                    wt = wosp.tile([128, KC, 512], BF16, tag=f"wos{m2}",
                                   name=f"wos{m2}t")
                    nc.sync.dma_start(out=wt[:], in_=wo4[m2])
                    wos.append(wt)
                for m2 in range(4):
                    for tt in range(BPC):
                        acc = psy.tile([128, 512], F32, tag="psy",
                                       name=f"y{m2}t{tt}")
                        for k in range(KC):
                            nc.tensor.matmul(
                                acc[:],
                                at[k][:, tt * 128:(tt + 1) * 128],
                                wos[m2][:, k, :],
                                start=(k == 0),
                                stop=(k == KC - 1),
                            )
                        ysb = ysp.tile([128, 512], F32, tag="ysb")
                        ys = slice(tt * 128, (tt + 1) * 128)
                        if m2 == 3 and tt == BPC - 1:
                            # final tile: split copy+DMA in half across
                            # engines/queues to shorten the drain tail
                            nc.vector.tensor_copy(ysb[:, :256], acc[:, :256])
                            nc.scalar.copy(out=ysb[:, 256:], in_=acc[:, 256:])
                            nc.gpsimd.dma_start(
                                out=y[ys, m2 * 512:m2 * 512 + 256],
                                in_=ysb[:, :256],
                            )
                            nc.sync.dma_start(
                                out=y[ys, m2 * 512 + 256:(m2 + 1) * 512],
                                in_=ysb[:, 256:],
                            )
                        else:
                            if tt % 2 == 0:
                                nc.vector.tensor_copy(ysb[:], acc[:])
                            else:
                                nc.scalar.copy(out=ysb[:], in_=acc[:])
                            nc.gpsimd.dma_start(
                                out=y[ys, m2 * 512:(m2 + 1) * 512],
                                in_=ysb[:],
                            )

    nc.compile()
    return nc


def _prep_inputs(x, freqs_cos, freqs_sin, wq, wk, wv, wo):
    x = np.asarray(x, dtype=np.float32)
    fc = np.asarray(freqs_cos, dtype=np.float32)
    fs = np.asarray(freqs_sin, dtype=np.float32)
    wq = np.asarray(wq, np.float32)
    wk = np.asarray(wk, np.float32)
    wv = np.asarray(wv, np.float32)
    wo = np.asarray(wo, np.float32)

    # head-dim permutation: even components to rows 0-63, odd to 64-127,
    # so the RoPE pair swap becomes a partition block swap (scores are
    # invariant since q and k use the same permutation)
    perm = np.concatenate([np.arange(0, 128, 2), np.arange(1, 128, 2)])
    shared = {
        # [n/m, p, kc, nn] = w[block*bw + perm[nn], kc*128 + p]
        "wq4": np.ascontiguousarray(
            wq.reshape(H, 128, KC, 128)[:, perm].transpose(0, 3, 2, 1)
        ).astype(BF),
        "wk4": np.ascontiguousarray(
            wk.reshape(H, 128, KC, 128)[:, perm].transpose(0, 3, 2, 1)
        ).astype(BF),
        "wv4": np.ascontiguousarray(
            wv.reshape(4, 512, KC, 128).transpose(0, 3, 2, 1)).astype(BF),
        "wo4": np.ascontiguousarray(
            wo.reshape(4, 512, KC, 128).transpose(0, 3, 2, 1)).astype(BF),
    }
    shared["ident"] = np.eye(128, dtype=np.float32).astype(BF)
    mask = np.triu(np.full((128, 128), -1e30, np.float32), k=1)
    shared["mask4"] = np.ascontiguousarray(np.tile(mask, (1, 4)))

    # permuted layout: rows 0-63 = even slots (sign -s), 64-127 = odd (+s)
    cosd = np.vstack([fc.T, fc.T])              # [128, 128]
    sind = np.vstack([-fs.T, fs.T])
    cos8 = np.ascontiguousarray(np.tile(cosd, (1, 8)))  # [128, 1024]
    sin8 = np.ascontiguousarray(np.tile(sind, (1, 8)))
    scale = np.float32(1.0 / np.sqrt(HD))
    shared["cosq"] = (cos8 * scale).astype(BF)
    shared["sinq"] = (sin8 * scale).astype(BF)
    shared["cosk"] = cos8.astype(BF)
    shared["sink"] = sin8.astype(BF)

    in_maps = []
    for i in range(N_CORES):
        shard = x[i * BPC:(i + 1) * BPC].reshape(TOK, C)
        m = dict(shared)
        m["xT"] = np.ascontiguousarray(shard.T).astype(BF)
        in_maps.append(m)
    return in_maps


def _run(inputs, trace=False):
    if "nc" not in _CACHE:
        _CACHE["nc"] = _build()
    nc = _CACHE["nc"]
    in_maps = _prep_inputs(**inputs)
    res = run_bass_kernel_spmd(
        nc, in_maps, core_ids=list(range(N_CORES)), trace=trace
    )
    out = np.empty((B, T, C), np.float32)
    for i in range(N_CORES):
        out[i * BPC:(i + 1) * BPC] = np.asarray(res.results[i]["y"]).reshape(
            BPC, T, C
        )
    return out, res


def kernel(**inputs):
    out, _ = _run(inputs, trace=False)
    return out
